# revision 12
# baseline (speedup 1.0000x reference)
"""Causal self-attention (B=2, S=2048, D=1024, H=16) on 8 TRN2 NeuronCores.

Sharding strategy (head-parallel + AllToAll):
  - Each core owns 2 heads (of 16). Wqkv is column-sharded per core (with the
    per-head q/k/v blocks regrouped host-side into [q_h0 q_h1 | k_h0 k_h1 |
    v_h0 v_h1] order so projection PSUM tiles evict straight into the q/k/vT
    SBUF layouts used by attention).
  - x is pre-transposed host-side to xT [D, B*S] so the projection reads it
    directly as the moving operand (contraction dim on partitions).
  - Projection computes qT/kT/vT [dims, seq]; scores are computed transposed
    (scoresT [keys, queries]) so softmax denominators come from a ones-column
    folded into the PV stationary operand, and the attention output attnT
    [dims, seq] is directly the stationary operand of the out-projection.
  - Softmax skips the max-subtraction: scores/8 for this problem's scale are
    bounded (|s| <~ 7), so exp never overflows and denominators stay in a
    healthy fp32 range.
  - One AllToAll converts the head-sharded attnT into a row-sharded layout;
    each core then computes its 512 output rows with full Wout and the host
    concatenates the 8 row-slices.

Compute dtype is bf16 (fp32 PSUM accumulation), matching the usual 2e-2
rel-err envelope for these kernels.
"""

import numpy as np
import ml_dtypes

import concourse.bass as bass
import concourse.mybir as mybir
import concourse.tile as tile
from concourse.masks import make_identity
from concourse.vector_clock import ScopedClock

N_CORES = 8
B, S_FULL, D = 2, 2048, 1024
H = 16
DH = 64
HPC = H // N_CORES  # heads per core
QT = 512  # query tile (moving free dim)
KT = 128  # key tile (psum partition dim)

BF16 = mybir.dt.bfloat16
F32 = mybir.dt.float32

# ---------------------------------------------------------------------------
# Patch: walrus in this toolchain rejects >1 sync-wait on a Drain (TPB_CTRL)
# instruction. Split the Tile kernel-tail drain's waits across a drain chain.
# ---------------------------------------------------------------------------


def _patched_drain_and_barrier(self, tick_clock, wait_clock):
    nc = self.nc
    drain_inst = nc.sync.drain()
    wait_clock.add_sem_waits(
        drain_inst.ins, ScopedClock({None: tick_clock.global_clock})
    )
    si = drain_inst.ins.sync_info
    if si is not None and si.on_wait and len(si.on_wait) > 1:
        waits = list(si.on_wait)
        drain_inst.ins.sync_info = mybir.SyncInfo(on_wait=[waits[0]], on_update=[])
        for w in waits[1:]:
            extra = nc.sync.drain()
            extra.ins.sync_info = mybir.SyncInfo(on_wait=[w], on_update=[])
    nc.all_engine_barrier()
    popped = nc._tile_sem_poison_stack.pop()
    assert popped is self._sem_poison
    nc.clear_and_free_semaphores(list(self.sems.allocated().values()))
    nc.all_engine_barrier()


if getattr(tile.TileContext._drain_and_barrier, "__name__", "") != (
    "_patched_drain_and_barrier"
):
    tile.TileContext._drain_and_barrier = _patched_drain_and_barrier


def _split_excess_waits(nc, limit=1):
    """Walrus here encodes at most `limit` sem-waits per instruction; hoist
    the rest onto standalone event-semaphore instructions on the same engine
    (the engine stalls on those first, preserving semantics)."""
    for bb in nc.main_func.blocks:
        new = []
        for ins in bb.instructions:
            si = ins.sync_info
            waits = list(si.on_wait) if si is not None and si.on_wait else []
            if len(waits) > limit:
                for w in waits[:-limit]:
                    ev = mybir.InstEventSemaphore(
                        name=f"I-{nc.next_id()}", ins=[], outs=[], engine=ins.engine
                    )
                    ev.sync_info = mybir.SyncInfo(on_wait=[w], on_update=[])
                    nc.register_instruction(ev)
                    new.append(ev)
                ins.sync_info = mybir.SyncInfo(
                    on_wait=waits[-limit:], on_update=list(si.on_update)
                )
            new.append(ins)
        bb.instructions = new


# ---------------------------------------------------------------------------
# Device graph
# ---------------------------------------------------------------------------


def build_nc(S=S_FULL):
    BS = B * S
    n_qt = S // QT  # query tiles per batch
    n_kt = S // KT  # key tiles per batch
    n_sc = BS // QT  # 512-wide seq chunks over both batches
    n_st = BS // KT  # 128-wide seq tiles over both batches
    rows = BS // N_CORES  # output rows per core
    n_m = rows // 128  # out-projection row tiles
    QKV = 3 * HPC * DH  # per-core projection width (384)

    nc = bass.Bass(num_devices=N_CORES)
    xt = nc.declare_dram_parameter("xt", [D, BS], BF16, isOutput=False)
    wqkv = nc.declare_dram_parameter("wqkv", [D, QKV], BF16, isOutput=False)
    bqkv = nc.declare_dram_parameter("bqkv", [QKV], F32, isOutput=False)
    wout = nc.declare_dram_parameter("wout", [D, D], BF16, isOutput=False)
    bout = nc.declare_dram_parameter("bout", [D], F32, isOutput=False)
    out = nc.declare_dram_parameter("out", [rows, D], F32, isOutput=True)

    cc_in = nc.dram_tensor("cc_in", [N_CORES, HPC, DH, rows], BF16)
    cc_out = nc.dram_tensor("cc_out", [N_CORES, HPC, DH, rows], BF16)
    # per-(dest,head) softmax-denominator reciprocal bounce (for partition bcast)
    rd_dram = nc.dram_tensor("rd_dram", [B * (S // QT), HPC, QT], F32)

    Exp = mybir.ActivationFunctionType.Exp

    from contextlib import ExitStack

    with tile.TileContext(nc) as tc, ExitStack() as ctx:
        const = ctx.enter_context(tc.tile_pool(name="const", bufs=1))
        xt_pool = ctx.enter_context(tc.tile_pool(name="xt_pool", bufs=3))
        pt_pool = ctx.enter_context(tc.tile_pool(name="pt_pool", bufs=12))
        attn_pool = ctx.enter_context(tc.tile_pool(name="attn_pool", bufs=4))
        misc_pool = ctx.enter_context(tc.tile_pool(name="misc_pool", bufs=3))
        out_pool = ctx.enter_context(tc.tile_pool(name="out_pool", bufs=3))
        ps_acc = ctx.enter_context(tc.tile_pool(name="ps_acc", bufs=4, space="PSUM"))
        ps_fast = ctx.enter_context(tc.tile_pool(name="ps_fast", bufs=3, space="PSUM"))

        if True:
            # ---- constants / persistent buffers ----
            wqkv_sb = const.tile([128, D // 128, QKV], BF16, name="wqkv_sb")
            nc.sync.dma_start(
                out=wqkv_sb, in_=wqkv.rearrange("(kt p) m -> p kt m", p=128)
            )
            bqkv_sb = const.tile([128, QKV // 128], F32, name="bqkv_sb")
            nc.sync.dma_start(
                out=bqkv_sb, in_=bqkv.rearrange("(m p) -> p m", p=128)
            )
            wout_sb = const.tile([128, D // 128, D], BF16, name="wout_sb")
            nc.sync.dma_start(
                out=wout_sb, in_=wout.rearrange("(kt p) n -> p kt n", p=128)
            )
            bout_bc = const.tile([128, D], F32, name="bout_bc")
            nc.sync.dma_start(
                out=bout_bc,
                in_=bout.rearrange("(a n) -> a n", a=1).to_broadcast((128, D)),
            )
            ident = const.tile([128, 128], BF16, name="ident")
            make_identity(nc, ident)

            q_sb = const.tile([128, BS], BF16, name="q_sb")
            k_sb = const.tile([128, BS], BF16, name="k_sb")
            vt_sb = const.tile([128, BS], BF16, name="vt_sb")
            # v in normal orientation, per 128-seq tile; per head 64 v-dims
            # followed by a ones column (for the softmax denominator) + pad.
            v_sb = const.tile([128, n_st, 132], BF16, name="v_sb")
            nc.vector.memset(v_sb[:, :, 64:65], 1.0)
            nc.vector.memset(v_sb[:, :, 130:131], 1.0)

            # ---- phase 1: qkv projection (transposed outputs) ----
            xt_r = xt.rearrange("(kt p) s -> p kt s", p=128)
            for sc in range(n_sc):
                xt_t = xt_pool.tile([128, D // 128, QT], BF16, name="xt_t")
                nc.sync.dma_start(
                    out=xt_t, in_=xt_r[:, :, sc * QT : (sc + 1) * QT]
                )
                for m, dst in ((0, q_sb), (1, k_sb), (2, vt_sb)):
                    ps = ps_acc.tile([128, QT], F32, name="ps_proj", tag="acc")
                    for kt in range(D // 128):
                        nc.tensor.matmul(
                            ps,
                            lhsT=wqkv_sb[:, kt, m * 128 : (m + 1) * 128],
                            rhs=xt_t[:, kt, :],
                            start=(kt == 0),
                            stop=(kt == D // 128 - 1),
                        )
                    nc.vector.tensor_add(
                        dst[:, sc * QT : (sc + 1) * QT],
                        ps,
                        bqkv_sb[:, m : m + 1].to_broadcast((128, QT)),
                    )

            # ---- phase 1.5: transpose vT -> v (normal orientation) ----
            for st in range(n_st):
                pst = ps_fast.tile([128, 128], BF16, name="ps_tr", tag="fast")
                nc.tensor.transpose(
                    pst, vt_sb[:, st * 128 : (st + 1) * 128], ident
                )
                nc.vector.tensor_copy(v_sb[:, st, 0:64], pst[:, 0:64])
                nc.vector.tensor_copy(v_sb[:, st, 66:130], pst[:, 64:128])

            # ---- phase 2: causal attention, transposed ----
            for bb in range(B):
                for qt in range(n_qt):
                    ci = bb * n_qt + qt  # chunk index for the recip bounce
                    q_glob = bb * S + qt * QT  # global flattened row offset
                    q_off = q_glob
                    n_kv = (qt + 1) * (QT // KT)
                    pv_ps = [
                        ps_acc.tile([128, QT], F32, name=f"ps_pv{h}", tag="acc")
                        for h in range(HPC)
                    ]
                    for kv in range(n_kv):
                        st_idx = bb * n_kt + kv
                        k_off = bb * S + kv * KT
                        for h in range(HPC):
                            ssp = ps_fast.tile(
                                [128, QT], F32, name="ps_score", tag="fast"
                            )
                            nc.tensor.matmul(
                                ssp,
                                lhsT=k_sb[64 * h : 64 * h + 64, k_off : k_off + KT],
                                rhs=q_sb[64 * h : 64 * h + 64, q_off : q_off + QT],
                                start=True,
                                stop=True,
                            )
                            pt = pt_pool.tile([128, QT], BF16, name="pt")
                            nc.scalar.activation(pt, ssp, Exp, scale=0.125)
                            delta = kv * KT - qt * QT
                            if delta >= 0:
                                # diagonal tile: zero out keys above the diagonal
                                nc.gpsimd.affine_select(
                                    out=pt,
                                    in_=pt,
                                    pattern=[[1, QT]],
                                    channel_multiplier=-1,
                                    base=-delta,
                                    compare_op=mybir.AluOpType.is_ge,
                                    fill=0.0,
                                )
                            nc.tensor.matmul(
                                pv_ps[h][0:65, :],
                                lhsT=v_sb[:, st_idx, 66 * h : 66 * h + 65],
                                rhs=pt,
                                start=(kv == 0),
                                stop=(kv == n_kv - 1),
                            )
                    for h in range(HPC):
                        rc = misc_pool.tile([128, QT], F32, name="rc")
                        nc.vector.reciprocal(rc[64:65, :], pv_ps[h][64:65, :])
                        nc.sync.dma_start(out=rd_dram[ci, h], in_=rc[64:65, :])
                        rb = misc_pool.tile([64, QT], F32, name="rb")
                        nc.sync.dma_start(
                            out=rb,
                            in_=rd_dram[ci, h]
                            .rearrange("(a q) -> a q", a=1)
                            .to_broadcast((64, QT)),
                        )
                        at = attn_pool.tile([64, QT], BF16, name="at")
                        nc.vector.tensor_mul(at, pv_ps[h][0:64, :], rb)
                        # scatter this q-chunk into the per-destination blocks
                        for off in range(0, QT, rows):
                            w = min(rows, QT - off)
                            d = (q_glob + off) // rows
                            loc = (q_glob + off) % rows
                            nc.sync.dma_start(
                                out=cc_in[d, h, :, loc : loc + w],
                                in_=at[:, off : off + w],
                            )

            # ---- phase 3: exchange + out-projection ----
            nc.gpsimd.collective_compute(
                "AllToAll",
                mybir.AluOpType.bypass,
                replica_groups=[list(range(N_CORES))],
                ins=[cc_in[:].opt()],
                outs=[cc_out[:].opt()],
            )
            ao_sb = const.tile([128, N_CORES, rows], BF16, name="ao_sb")
            nc.sync.dma_start(out=ao_sb, in_=cc_out.rearrange("s h p q -> (h p) s q"))
            for m in range(n_m):
                for n in range(D // QT):
                    pso = ps_acc.tile([128, QT], F32, name="ps_out", tag="acc")
                    for s8 in range(N_CORES):
                        nc.tensor.matmul(
                            pso,
                            lhsT=ao_sb[:, s8, m * 128 : (m + 1) * 128],
                            rhs=wout_sb[:, s8, n * QT : (n + 1) * QT],
                            start=(s8 == 0),
                            stop=(s8 == N_CORES - 1),
                        )
                    osb = out_pool.tile([128, QT], F32, name="osb")
                    nc.vector.tensor_add(osb, pso, bout_bc[:, n * QT : (n + 1) * QT])
                    nc.sync.dma_start(
                        out=out[m * 128 : (m + 1) * 128, n * QT : (n + 1) * QT],
                        in_=osb,
                    )
    _split_excess_waits(nc)
    return nc


# ---------------------------------------------------------------------------
# Host side
# ---------------------------------------------------------------------------

_NC_CACHE = {}


def _get_nc(S=S_FULL):
    if S not in _NC_CACHE:
        _NC_CACHE[S] = build_nc(S)
    return _NC_CACHE[S]


def make_in_maps(x, Wqkv, bqkv, Wout, bout):
    """Shard/replicate full inputs into the 8 per-core input dicts."""
    x = np.asarray(x, dtype=np.float32)
    Wqkv = np.asarray(Wqkv, dtype=np.float32)
    bqkv = np.asarray(bqkv, dtype=np.float32)
    Wout = np.asarray(Wout, dtype=np.float32)
    bout = np.asarray(bout, dtype=np.float32)
    b, s, d = x.shape

    xt = np.ascontiguousarray(x.reshape(b * s, d).T).astype(ml_dtypes.bfloat16)
    wout_b = Wout.astype(ml_dtypes.bfloat16)
    in_maps = []
    for c in range(N_CORES):
        blocks = []
        for part in range(3):  # q, k, v
            for h in (HPC * c, HPC * c + 1):
                base = h * 3 * DH + part * DH
                blocks.append(np.arange(base, base + DH))
        idx = np.concatenate(blocks)
        in_maps.append(
            {
                "xt": xt,
                "wqkv": Wqkv[:, idx].astype(ml_dtypes.bfloat16),
                "bqkv": np.ascontiguousarray(bqkv[idx]),
                "wout": wout_b,
                "bout": bout,
            }
        )
    return in_maps


def kernel(x, Wqkv, bqkv, Wout, bout):
    from concourse.bass_utils import run_bass_kernel_spmd

    x = np.asarray(x, dtype=np.float32)
    b, s, d = x.shape
    nc = _get_nc(s)
    in_maps = make_in_maps(x, Wqkv, bqkv, Wout, bout)
    res = run_bass_kernel_spmd(nc, in_maps, core_ids=list(range(N_CORES)))
    full = np.concatenate(
        [np.asarray(res.results[c]["out"], dtype=np.float32) for c in range(N_CORES)],
        axis=0,
    )
    return full.reshape(b, s, d)


# revision 36
# speedup vs baseline: 9312.1108x; 9312.1108x over previous
"""Causal self-attention (B=2, S=2048, D=1024, H=16) on 8 TRN2 NeuronCores.

Sharding strategy (head-parallel + AllToAll):
  - Each core owns 2 heads (of 16). Wqkv is column-sharded per core (with the
    per-head q/k/v blocks regrouped host-side into [q_h0 q_h1 | k_h0 k_h1 |
    v_h0 v_h1] order so projection PSUM tiles evict straight into the q/k/vT
    SBUF layouts used by attention).
  - x is pre-transposed host-side to xT [D, B*S] so the projection reads it
    directly as the moving operand (contraction dim on partitions).
  - Projection computes qT/kT/vT [dims, seq]; scores are computed transposed
    (scoresT [keys, queries]) so softmax denominators come from a ones-column
    folded into the PV stationary operand, and the attention output attnT
    [dims, seq] is directly the stationary operand of the out-projection.
  - Softmax skips the max-subtraction: scores/8 for this problem's scale are
    bounded (|s| <~ 7), so exp never overflows and denominators stay in a
    healthy fp32 range.
  - One AllToAll converts the head-sharded attnT into a row-sharded layout;
    each core then computes its 512 output rows with full Wout and the host
    concatenates the 8 row-slices.

Compute dtype is bf16 (fp32 PSUM accumulation), matching the usual 2e-2
rel-err envelope for these kernels.
"""

import numpy as np
import ml_dtypes

import concourse.bass as bass
import concourse.mybir as mybir
import concourse.tile as tile
from concourse.masks import make_identity
from concourse.vector_clock import ScopedClock

N_CORES = 8
B, S_FULL, D = 2, 2048, 1024
H = 16
DH = 64
HPC = H // N_CORES  # heads per core
QT = 512  # query tile (moving free dim)
KT = 128  # key tile (psum partition dim)

BF16 = mybir.dt.bfloat16
F32 = mybir.dt.float32

# ---------------------------------------------------------------------------
# Patch: walrus in this toolchain rejects >1 sync-wait on a Drain (TPB_CTRL)
# instruction. Split the Tile kernel-tail drain's waits across a drain chain.
# ---------------------------------------------------------------------------


def _patched_drain_and_barrier(self, tick_clock, wait_clock):
    nc = self.nc
    drain_inst = nc.sync.drain()
    wait_clock.add_sem_waits(
        drain_inst.ins, ScopedClock({None: tick_clock.global_clock})
    )
    si = drain_inst.ins.sync_info
    if si is not None and si.on_wait and len(si.on_wait) > 1:
        waits = list(si.on_wait)
        drain_inst.ins.sync_info = mybir.SyncInfo(on_wait=[waits[0]], on_update=[])
        for w in waits[1:]:
            extra = nc.sync.drain()
            extra.ins.sync_info = mybir.SyncInfo(on_wait=[w], on_update=[])
    nc.all_engine_barrier()
    popped = nc._tile_sem_poison_stack.pop()
    assert popped is self._sem_poison
    nc.clear_and_free_semaphores(list(self.sems.allocated().values()))
    nc.all_engine_barrier()


if getattr(tile.TileContext._drain_and_barrier, "__name__", "") != (
    "_patched_drain_and_barrier"
):
    tile.TileContext._drain_and_barrier = _patched_drain_and_barrier


def _split_excess_waits(nc, limit=1):
    """Walrus here encodes at most `limit` sem-waits per instruction; hoist
    the rest onto standalone event-semaphore instructions on the same engine
    (the engine stalls on those first, preserving semantics)."""
    for bb in nc.main_func.blocks:
        new = []
        for ins in bb.instructions:
            si = ins.sync_info
            waits = list(si.on_wait) if si is not None and si.on_wait else []
            if len(waits) > limit:
                for w in waits[:-limit]:
                    ev = mybir.InstEventSemaphore(
                        name=f"I-{nc.next_id()}", ins=[], outs=[], engine=ins.engine
                    )
                    ev.sync_info = mybir.SyncInfo(on_wait=[w], on_update=[])
                    nc.register_instruction(ev)
                    new.append(ev)
                ins.sync_info = mybir.SyncInfo(
                    on_wait=waits[-limit:], on_update=list(si.on_update)
                )
            new.append(ins)
        bb.instructions = new


# ---------------------------------------------------------------------------
# Device graph
# ---------------------------------------------------------------------------


def build_nc(S=S_FULL):
    BS = B * S
    n_qt = S // QT  # query tiles per batch
    n_kt = S // KT  # key tiles per batch
    n_sc = BS // QT  # 512-wide seq chunks over both batches
    n_st = BS // KT  # 128-wide seq tiles over both batches
    rows = BS // N_CORES  # output rows per core
    n_m = rows // 128  # out-projection row tiles
    QKV = 3 * HPC * DH  # per-core projection width (384)

    nc = bass.Bass(num_devices=N_CORES)
    xt = nc.declare_dram_parameter("xt", [D, BS], BF16, isOutput=False)
    wqkv = nc.declare_dram_parameter("wqkv", [D, QKV], BF16, isOutput=False)
    bqkv = nc.declare_dram_parameter("bqkv", [QKV], F32, isOutput=False)
    wout = nc.declare_dram_parameter("wout", [D, D], BF16, isOutput=False)
    bout = nc.declare_dram_parameter("bout", [D], F32, isOutput=False)
    out = nc.declare_dram_parameter("out", [rows, D], F32, isOutput=True)

    cc_in = nc.dram_tensor("cc_in", [N_CORES, HPC, DH, rows], BF16)
    cc_out = nc.dram_tensor("cc_out", [N_CORES, HPC, DH, rows], BF16)

    Exp = mybir.ActivationFunctionType.Exp

    from contextlib import ExitStack

    with tile.TileContext(nc) as tc, ExitStack() as ctx:
        const = ctx.enter_context(tc.tile_pool(name="const", bufs=1))
        xt_pool = ctx.enter_context(tc.tile_pool(name="xt_pool", bufs=3))
        pt_pool = ctx.enter_context(tc.tile_pool(name="pt_pool", bufs=16))
        attn_pool = ctx.enter_context(tc.tile_pool(name="attn_pool", bufs=6))
        misc_pool = ctx.enter_context(tc.tile_pool(name="misc_pool", bufs=4))
        out_pool = ctx.enter_context(tc.tile_pool(name="out_pool", bufs=3))
        # PSUM (8 banks of [128, 2KB]): scores pairs 2 banks x 2 bufs = 4,
        # pv accumulators 2, misc (proj/outproj/recip-bcast) 2.
        ps_sc = ctx.enter_context(tc.tile_pool(name="ps_sc", bufs=2, space="PSUM"))
        ps_pv = ctx.enter_context(tc.tile_pool(name="ps_pv", bufs=2, space="PSUM"))
        ps_misc = ctx.enter_context(tc.tile_pool(name="ps_misc", bufs=2, space="PSUM"))

        if True:
            # ---- constants / persistent buffers ----
            wqkv_sb = const.tile([128, D // 128, QKV], BF16, name="wqkv_sb")
            nc.sync.dma_start(
                out=wqkv_sb, in_=wqkv.rearrange("(kt p) m -> p kt m", p=128)
            )
            bqkv_sb = const.tile([128, QKV // 128], F32, name="bqkv_sb")
            nc.sync.dma_start(
                out=bqkv_sb, in_=bqkv.rearrange("(m p) -> p m", p=128)
            )
            ident = const.tile([128, 128], BF16, name="ident")
            make_identity(nc, ident)

            q_sb = const.tile([128, BS], BF16, name="q_sb")
            k_sb = const.tile([128, BS], BF16, name="k_sb")
            vt_sb = const.tile([128, BS], BF16, name="vt_sb")
            # v in normal orientation, per 128-seq tile; per head 64 v-dims
            # followed by a ones column (for the softmax denominator) + pad.
            v_sb = const.tile([128, n_st, 132], BF16, name="v_sb")
            nc.vector.memset(v_sb[:, :, 64:65], 1.0)
            nc.vector.memset(v_sb[:, :, 130:131], 1.0)
            # fp16 ones row at partition 64 (stationary of the recip-broadcast
            # matmul; its data row must share the partition of the recip row)
            ones_r = const.tile([128, 64], mybir.dt.float16, name="ones_r")
            nc.vector.memset(ones_r[64:65, :], 1.0)

            # ---- phase 1: qkv projection (transposed outputs) ----
            xt_r = xt.rearrange("(kt p) s -> p kt s", p=128)

            def proj_chunk(sc):
                xt_t = xt_pool.tile([128, D // 128, QT], BF16, name="xt_t")
                if sc == 0:
                    # split the first chunk per k-tile so the first matmul can
                    # start as soon as k-tile 0 lands
                    for kt in range(D // 128):
                        nc.sync.dma_start(
                            out=xt_t[:, kt, :],
                            in_=xt_r[:, kt, 0:QT],
                        )
                else:
                    nc.sync.dma_start(
                        out=xt_t, in_=xt_r[:, :, sc * QT : (sc + 1) * QT]
                    )
                for m, dst in ((0, q_sb), (1, k_sb), (2, vt_sb)):
                    ps = ps_misc.tile([128, QT], F32, name="ps_proj", tag="misc")
                    for kt in range(D // 128):
                        nc.tensor.matmul(
                            ps,
                            lhsT=wqkv_sb[:, kt, m * 128 : (m + 1) * 128],
                            rhs=xt_t[:, kt, :],
                            start=(kt == 0),
                            stop=(kt == D // 128 - 1),
                        )
                    nc.vector.tensor_add(
                        dst[:, sc * QT : (sc + 1) * QT],
                        ps,
                        bqkv_sb[:, m : m + 1].to_broadcast((128, QT)),
                    )
                # transpose this chunk's vT -> v (normal orientation)
                for st in range(sc * (QT // KT), (sc + 1) * (QT // KT)):
                    pst = ps_sc.tile([128, 128], BF16, name="ps_tr", tag="sc")
                    nc.tensor.transpose(
                        pst, vt_sb[:, st * 128 : (st + 1) * 128], ident
                    )
                    nc.vector.tensor_copy(v_sb[:, st, 0:64], pst[:, 0:64])
                    nc.vector.tensor_copy(v_sb[:, st, 66:130], pst[:, 64:128])

            # ---- phase 2: causal attention, transposed ----
            def att_chunk(bb, qt):
                if True:
                    q_glob = bb * S + qt * QT  # global flattened row offset
                    q_off = q_glob
                    n_kv = (qt + 1) * (QT // KT)
                    pv_ps = [
                        ps_pv.tile([128, QT], F32, name=f"ps_pv{h}", tag="pv")
                        for h in range(HPC)
                    ]
                    for kv in range(n_kv):
                        st_idx = bb * n_kt + kv
                        k_off = bb * S + kv * KT
                        delta = kv * KT - qt * QT
                        # columns [0:delta) of this q-tile are entirely masked
                        # for this kv tile: trim scores/exp/mask/PV to [c0:QT)
                        c0 = max(delta, 0)
                        W = QT - c0
                        # both heads' scoresT into one 2-bank psum pair; the
                        # two matmuls are row-tiled ((0,0)/(64,0)) and overlap
                        # in the PE array
                        ssp = ps_sc.tile([128, HPC, QT], F32, name="ps_score",
                                         tag="sc")
                        for h in range(HPC):
                            nc.tensor.matmul(
                                ssp[:, h, c0:QT],
                                lhsT=k_sb[64 * h : 64 * h + 64, k_off : k_off + KT],
                                rhs=q_sb[
                                    64 * h : 64 * h + 64,
                                    q_off + c0 : q_off + QT,
                                ],
                                start=True,
                                stop=True,
                            )
                        pt = pt_pool.tile([128, HPC, QT], BF16, name="pt")
                        nc.scalar.activation(
                            pt[:, :, c0:QT], ssp[:, :, c0:QT], Exp, scale=0.125
                        )
                        if delta >= 0:
                            # diagonal tile: zero out keys above the diagonal
                            # (head dim iota step 0: same mask for both heads;
                            # in trimmed coords keep iff (i - j) >= 0)
                            nc.gpsimd.affine_select(
                                out=pt[:, :, c0:QT],
                                in_=pt[:, :, c0:QT],
                                pattern=[[0, HPC], [1, W]],
                                channel_multiplier=-1,
                                base=0,
                                compare_op=mybir.AluOpType.is_ge,
                                fill=0.0,
                            )
                        for h in range(HPC):
                            nc.tensor.matmul(
                                pv_ps[h][0:65, c0:QT],
                                lhsT=v_sb[:, st_idx, 66 * h : 66 * h + 65],
                                rhs=pt[:, h, c0:QT],
                                start=(kv == 0),
                                stop=(kv == n_kv - 1),
                            )
                    for h in range(HPC):
                        # denominator reciprocal, broadcast across the 64
                        # attn partitions via a K=1 f32r matmul (ones row)
                        rc = misc_pool.tile(
                            [128, QT], mybir.dt.float16, name="rc"
                        )
                        with nc.allow_low_precision(
                            reason="fp16 recip row feeding broadcast matmul"
                        ):
                            nc.vector.reciprocal(
                                rc[64:65, :], pv_ps[h][64:65, :]
                            )
                        rbp = ps_misc.tile([64, QT], F32, name="rbp", tag="misc")
                        nc.tensor.matmul(
                            rbp,
                            lhsT=ones_r[64:65, :],
                            rhs=rc[64:65, :],
                            start=True,
                            stop=True,
                        )
                        rb = misc_pool.tile([64, QT], F32, name="rb")
                        nc.vector.tensor_copy(rb, rbp)
                        at = attn_pool.tile([64, QT], BF16, name="at")
                        nc.vector.tensor_mul(at, pv_ps[h][0:64, :], rb)
                        # scatter this q-chunk into the per-destination blocks
                        for off in range(0, QT, rows):
                            w = min(rows, QT - off)
                            d = (q_glob + off) // rows
                            loc = (q_glob + off) % rows
                            nc.sync.dma_start(
                                out=cc_in[d, h, :, loc : loc + w],
                                in_=at[:, off : off + w],
                            )

            for sc in range(n_sc):
                proj_chunk(sc)
            for sc in range(n_sc):
                att_chunk(sc // n_qt, sc % n_qt)

            # ---- phase 3: exchange + out-projection ----
            wout_sb = const.tile([128, D // 128, D], BF16, name="wout_sb")
            nc.sync.dma_start(
                out=wout_sb, in_=wout.rearrange("(kt p) n -> p kt n", p=128)
            )
            bout_bc = const.tile([128, D], F32, name="bout_bc")
            nc.sync.dma_start(
                out=bout_bc,
                in_=bout.rearrange("(a n) -> a n", a=1).to_broadcast((128, D)),
            )
            nc.gpsimd.collective_compute(
                "AllToAll",
                mybir.AluOpType.bypass,
                replica_groups=[list(range(N_CORES))],
                ins=[cc_in[:].opt()],
                outs=[cc_out[:].opt()],
            )
            ao_sb = const.tile([128, N_CORES, rows], BF16, name="ao_sb")
            cc_out_r = cc_out.rearrange("s h p q -> (h p) s q")
            for s8 in range(N_CORES):
                nc.sync.dma_start(out=ao_sb[:, s8, :], in_=cc_out_r[:, s8, :])
            for m in range(n_m):
                for n in range(D // QT):
                    pso = ps_misc.tile([128, QT], F32, name="ps_out", tag="misc")
                    for s8 in range(N_CORES):
                        nc.tensor.matmul(
                            pso,
                            lhsT=ao_sb[:, s8, m * 128 : (m + 1) * 128],
                            rhs=wout_sb[:, s8, n * QT : (n + 1) * QT],
                            start=(s8 == 0),
                            stop=(s8 == N_CORES - 1),
                        )
                    osb = out_pool.tile([128, QT], F32, name="osb")
                    nc.vector.tensor_add(osb, pso, bout_bc[:, n * QT : (n + 1) * QT])
                    nc.sync.dma_start(
                        out=out[m * 128 : (m + 1) * 128, n * QT : (n + 1) * QT],
                        in_=osb,
                    )
    _split_excess_waits(nc)
    return nc


# ---------------------------------------------------------------------------
# Host side
# ---------------------------------------------------------------------------

_NC_CACHE = {}


def _get_nc(S=S_FULL):
    if S not in _NC_CACHE:
        _NC_CACHE[S] = build_nc(S)
    return _NC_CACHE[S]


def make_in_maps(x, Wqkv, bqkv, Wout, bout):
    """Shard/replicate full inputs into the 8 per-core input dicts."""
    x = np.asarray(x, dtype=np.float32)
    Wqkv = np.asarray(Wqkv, dtype=np.float32)
    bqkv = np.asarray(bqkv, dtype=np.float32)
    Wout = np.asarray(Wout, dtype=np.float32)
    bout = np.asarray(bout, dtype=np.float32)
    b, s, d = x.shape

    xt = np.ascontiguousarray(x.reshape(b * s, d).T).astype(ml_dtypes.bfloat16)
    wout_b = Wout.astype(ml_dtypes.bfloat16)
    in_maps = []
    for c in range(N_CORES):
        blocks = []
        for part in range(3):  # q, k, v
            for h in (HPC * c, HPC * c + 1):
                base = h * 3 * DH + part * DH
                blocks.append(np.arange(base, base + DH))
        idx = np.concatenate(blocks)
        in_maps.append(
            {
                "xt": xt,
                "wqkv": Wqkv[:, idx].astype(ml_dtypes.bfloat16),
                "bqkv": np.ascontiguousarray(bqkv[idx]),
                "wout": wout_b,
                "bout": bout,
            }
        )
    return in_maps


def kernel(x, Wqkv, bqkv, Wout, bout):
    from concourse.bass_utils import run_bass_kernel_spmd

    x = np.asarray(x, dtype=np.float32)
    b, s, d = x.shape
    nc = _get_nc(s)
    in_maps = make_in_maps(x, Wqkv, bqkv, Wout, bout)
    res = run_bass_kernel_spmd(nc, in_maps, core_ids=list(range(N_CORES)))
    full = np.concatenate(
        [np.asarray(res.results[c]["out"], dtype=np.float32) for c in range(N_CORES)],
        axis=0,
    )
    return full.reshape(b, s, d)
